# revision 1
# baseline (speedup 1.0000x reference)
"""GroupedQueryAttention (B=1, T=4096, D=2048, 16 heads / 4 kv heads, RoPE,
causal) on 8 Trainium2 NeuronCores.

Sharding: head tensor-parallel. Core c owns q-heads {2c, 2c+1} and kv head
c//2 (WQ/WO split along head dim, WK/WV along kv-head dim). Each core
computes its partial out = ctx_heads @ WO_slice over the full sequence;
partials are summed across cores (all-reduce equivalent done on the host
gather side).

On-chip layout: activations live transposed (QT/KT = [d_head, T]) so every
matmul contracts over the partition dim; V stays natural [T, d_head] (it is
the AV lhsT). Softmax runs without max-subtraction (scores are O(+-8) here),
the denominator comes from a ones-column matmul (partition-dim reduction on
the PE), and causal masking is a post-exp multiply by 0/1 masks on the
diagonal tiles. RoPE is applied in transposed layout with a pair-swap
permutation matmul. Matmul operands are bf16 (fp32 PSUM accumulation,
fp32 RoPE/normalization arithmetic).
"""

import math

import numpy as np
import ml_dtypes

import concourse.bass as bass
import concourse.mybir as mybir
import concourse.tile as tile
from concourse.bass_utils import run_bass_kernel_spmd

FP = mybir.dt.float32
BF = mybir.dt.bfloat16
BFNP = ml_dtypes.bfloat16

T, D, DH = 4096, 2048, 128      # seq len, model dim, head dim
CH = 512                        # query-chunk (free dim of attention matmuls)
N_CORES = 8


# --------------------------------------------------------------------------
# workaround: this walrus build rejects instructions carrying >1 sem-waits
# (setupSyncWait "Too many sync wait commands"); split extras into NoOps.
_WS_CTR = [0]


def _split_multi_waits(nc, limit=1):
    for f in nc.m.functions:
        for bb in f.blocks:
            il = bb.instructions
            i = 0
            while i < len(il):
                inst = il[i]
                si = getattr(inst, "sync_info", None)
                if si is not None and len(si.on_wait) > limit:
                    waits = list(si.on_wait)
                    keep, rest = waits[:limit], waits[limit:]
                    nops = []
                    for j in range(0, len(rest), limit):
                        _WS_CTR[0] += 1
                        n = mybir.InstNoOp(name=f"waitsplit-{_WS_CTR[0]}")
                        n.engine = inst.engine
                        n.sync_info = mybir.SyncInfo(
                            on_wait=rest[j:j + limit], on_update=[])
                        nops.append(n)
                    inst.sync_info = mybir.SyncInfo(
                        on_wait=keep, on_update=list(si.on_update))
                    for k, n in enumerate(nops):
                        il.insert(i + k, n)
                    i += len(nops)
                i += 1


# --------------------------------------------------------------------------
def build_nc():
    nT = T // CH          # 8 T-chunks
    nA = D // 128         # 16 contraction tiles
    nS = CH // 128        # 4 q-subtiles per chunk
    nN = D // 512         # 4 output column tiles
    ISQ = 1.0 / math.sqrt(float(DH))

    nc = bass.Bass()

    xT = nc.dram_tensor("xT", [D, T], BF, kind="ExternalInput")
    wq2 = nc.dram_tensor("wq2", [128, nA * 256], BF, kind="ExternalInput")
    wk2 = nc.dram_tensor("wk2", [128, nA * 128], BF, kind="ExternalInput")
    wv2 = nc.dram_tensor("wv2", [128, nA * 128], BF, kind="ExternalInput")
    wo2 = nc.dram_tensor("wo2", [128, 2 * D], BF, kind="ExternalInput")
    cosT = nc.dram_tensor("cosT", [128, T], FP, kind="ExternalInput")
    sinT = nc.dram_tensor("sinT", [128, T], FP, kind="ExternalInput")
    permM = nc.dram_tensor("permM", [128, 128], FP, kind="ExternalInput")
    masks = nc.dram_tensor("masks", [128, nS * CH], BF, kind="ExternalInput")
    onescol = nc.dram_tensor("onescol", [128, 1], BF, kind="ExternalInput")
    onesrow = nc.dram_tensor("onesrow", [1, 128], FP, kind="ExternalInput")
    out = nc.dram_tensor("out", [T, D], FP, kind="ExternalOutput")

    xTr = xT.rearrange("(a p) t -> p a t", p=128)

    with tile.TileContext(nc) as tc:
        with (
            tc.tile_pool(name="res", bufs=1) as res,
            tc.tile_pool(name="ktv", bufs=2 * nT) as ktv,
            tc.tile_pool(name="xt", bufs=18) as xtp,
            tc.tile_pool(name="tab", bufs=4) as tab,
            tc.tile_pool(name="work", bufs=2) as work,
            tc.tile_pool(name="pp", bufs=3) as pp,
            tc.tile_pool(name="oo", bufs=4) as oo,
            tc.tile_pool(name="psA", bufs=2, space="PSUM") as psA,
            tc.tile_pool(name="psS", bufs=2, space="PSUM") as psS,
            tc.tile_pool(name="psC", bufs=2, space="PSUM") as psC,
            tc.tile_pool(name="psL", bufs=2, space="PSUM") as psL,
        ):
            wq_sb = res.tile([128, nA * 256], BF, name="wq_sb")
            wk_sb = res.tile([128, nA * 128], BF, name="wk_sb")
            wv_sb = res.tile([128, nA * 128], BF, name="wv_sb")
            wo_sb = res.tile([128, 2 * D], BF, name="wo_sb")
            mask_sb = res.tile([128, nS * CH], BF, name="mask_sb")
            perm_sb = res.tile([128, 128], FP, name="perm_sb")
            oc_sb = res.tile([128, 1], BF, name="oc_sb")
            or_sb = res.tile([1, 128], FP, name="or_sb")
            for t_, s_ in [
                (wq_sb, wq2), (wk_sb, wk2), (wv_sb, wv2), (wo_sb, wo2),
                (mask_sb, masks), (perm_sb, permM), (oc_sb, onescol),
                (or_sb, onesrow),
            ]:
                nc.sync.dma_start(t_[:], s_[:])

            kt_tiles = []
            v_tiles = []
            copyflip = [0]

            def copy_out(dst, src):
                if copyflip[0] % 2 == 0:
                    nc.vector.tensor_copy(dst, src)
                else:
                    nc.scalar.copy(dst, src)
                copyflip[0] += 1

            for qc in range(nT):
                t0 = qc * CH
                cos_t = tab.tile([128, CH], FP, name=f"cos{qc}", tag="tab")
                sin_t = tab.tile([128, CH], FP, name=f"sin{qc}", tag="tab")
                nc.sync.dma_start(cos_t[:], cosT[:, t0:t0 + CH])
                nc.sync.dma_start(sin_t[:], sinT[:, t0:t0 + CH])

                xts = []
                for a in range(nA):
                    xa = xtp.tile([128, CH], BF, name=f"x{qc}_{a}", tag="xt")
                    nc.sync.dma_start(xa[:], xTr[:, a, t0:t0 + CH])
                    xts.append(xa)

                def accum(lhs_slices, rhs_list, N, nm):
                    acc = psA.tile([128, N], FP, name=f"ps_{nm}", tag="acc")
                    for a in range(nA):
                        nc.tensor.matmul(
                            acc[:], lhs_slices(a), rhs_list(a),
                            start=(a == 0), stop=(a == nA - 1),
                        )
                    return acc

                q0_ps = accum(lambda a: wq_sb[:, a * 256:a * 256 + 128],
                              lambda a: xts[a][:], CH, f"q0_{qc}")
                q0_sb = work.tile([128, CH], FP, name=f"q0s{qc}", tag="qk",
                                  bufs=4)
                copy_out(q0_sb[:], q0_ps[:])

                q1_ps = accum(lambda a: wq_sb[:, a * 256 + 128:a * 256 + 256],
                              lambda a: xts[a][:], CH, f"q1_{qc}")
                q1_sb = work.tile([128, CH], FP, name=f"q1s{qc}", tag="qk",
                                  bufs=4)
                copy_out(q1_sb[:], q1_ps[:])

                k_ps = accum(lambda a: wk_sb[:, a * 128:(a + 1) * 128],
                             lambda a: xts[a][:], CH, f"k_{qc}")
                k_sb = work.tile([128, CH], FP, name=f"ks{qc}", tag="qk",
                                 bufs=4)
                copy_out(k_sb[:], k_ps[:])

                vt = ktv.tile([128, nS * 128], BF, name=f"v{qc}", tag="ktv")
                for s in range(nS):
                    v_ps = psA.tile([128, 128], FP, name=f"ps_v{qc}_{s}",
                                    tag="acc", padded_shape=[128, 512])
                    for a in range(nA):
                        nc.tensor.matmul(
                            v_ps[:], xts[a][:, s * 128:(s + 1) * 128],
                            wv_sb[:, a * 128:(a + 1) * 128],
                            start=(a == 0), stop=(a == nA - 1),
                        )
                    copy_out(vt[:, s * 128:(s + 1) * 128], v_ps[:])
                v_tiles.append(vt)

                def rope(src_sb, dst, nm):
                    sw = psS.tile([128, CH], FP, name=f"sw_{nm}", tag="s")
                    nc.tensor.matmul(sw[:], perm_sb[:], src_sb[:])
                    t1 = work.tile([128, CH], FP, name=f"r1_{nm}", tag="rt",
                                   bufs=4)
                    nc.vector.tensor_mul(t1[:], src_sb[:], cos_t[:])
                    t2 = work.tile([128, CH], FP, name=f"r2_{nm}", tag="rt",
                                   bufs=4)
                    nc.vector.tensor_mul(t2[:], sw[:], sin_t[:])
                    nc.vector.tensor_add(dst, t1[:], t2[:])

                qr0 = work.tile([128, CH], BF, name=f"qr0_{qc}", tag="qr",
                                bufs=4)
                qr1 = work.tile([128, CH], BF, name=f"qr1_{qc}", tag="qr",
                                bufs=4)
                ktt = ktv.tile([128, CH], BF, name=f"kt{qc}", tag="ktv")
                rope(q0_sb, qr0[:], f"q0_{qc}")
                rope(q1_sb, qr1[:], f"q1_{qc}")
                rope(k_sb, ktt[:], f"k_{qc}")
                kt_tiles.append(ktt)

                nkt = (qc + 1) * nS
                ctxn = []
                for h, qr in enumerate([qr0, qr1]):
                    ctx = psC.tile([128, CH], FP, name=f"ctx{qc}_{h}",
                                   tag="ctx")
                    lp = psL.tile([1, CH], FP, name=f"l{qc}_{h}", tag="l")
                    for kt in range(nkt):
                        kc, ko = kt // nS, (kt % nS) * 128
                        S = psS.tile([128, CH], FP, name=f"S{qc}_{h}_{kt}",
                                     tag="s")
                        nc.tensor.matmul(
                            S[:], kt_tiles[kc][:, ko:ko + 128], qr[:])
                        P = pp.tile([128, CH], BF, name=f"P{qc}_{h}_{kt}",
                                    tag="p")
                        nc.scalar.activation(
                            P[:], S[:], mybir.ActivationFunctionType.Exp,
                            scale=ISQ)
                        delta = kt - qc * nS
                        if delta >= 0:
                            nc.vector.tensor_mul(
                                P[:], P[:],
                                mask_sb[:, delta * CH:(delta + 1) * CH])
                        nc.tensor.matmul(
                            ctx[:], v_tiles[kc][:, ko:ko + 128], P[:],
                            start=(kt == 0), stop=(kt == nkt - 1))
                        nc.tensor.matmul(
                            lp[:], oc_sb[:], P[:],
                            start=(kt == 0), stop=(kt == nkt - 1))
                    r_sb = work.tile([1, CH], FP, name=f"r{qc}_{h}", tag="r")
                    nc.vector.reciprocal(r_sb[:], lp[:])
                    bc_ps = psS.tile([128, CH], FP, name=f"bc{qc}_{h}",
                                     tag="s")
                    nc.tensor.matmul(bc_ps[:], or_sb[:], r_sb[:])
                    bc_sb = work.tile([128, CH], FP, name=f"bcs{qc}_{h}",
                                      tag="bc")
                    nc.scalar.copy(bc_sb[:], bc_ps[:])
                    cn = work.tile([128, CH], BF, name=f"cn{qc}_{h}", tag="cn",
                                   bufs=4)
                    nc.vector.tensor_mul(cn[:], ctx[:], bc_sb[:])
                    ctxn.append(cn)

                for s in range(nS):
                    for n in range(nN):
                        w_ps = psA.tile([128, 512], FP, name=f"w{qc}_{s}_{n}",
                                        tag="acc")
                        nc.tensor.matmul(
                            w_ps[:], ctxn[0][:, s * 128:(s + 1) * 128],
                            wo_sb[:, n * 512:(n + 1) * 512],
                            start=True, stop=False)
                        nc.tensor.matmul(
                            w_ps[:], ctxn[1][:, s * 128:(s + 1) * 128],
                            wo_sb[:, D + n * 512:D + (n + 1) * 512],
                            start=False, stop=True)
                        osb = oo.tile([128, 512], FP, name=f"o{qc}_{s}_{n}",
                                      tag="osb")
                        copy_out(osb[:], w_ps[:])
                        nc.sync.dma_start(
                            out[t0 + s * 128:t0 + (s + 1) * 128,
                                n * 512:(n + 1) * 512], osb[:])

    _split_multi_waits(nc, 1)
    return nc


# --------------------------------------------------------------------------
def host_prep(x, WQ, WK, WV, WO):
    nA = D // 128
    nS = CH // 128
    ROPE_BASE = 10000.0

    xTc = np.ascontiguousarray(
        np.asarray(x, dtype=np.float32).reshape(T, D).T).astype(BFNP)

    omega = 1.0 / (ROPE_BASE ** (np.arange(0, DH, 2, dtype=np.float64) / DH))
    ang = np.outer(omega, np.arange(T, dtype=np.float64))
    cosT = np.repeat(np.cos(ang), 2, axis=0).astype(np.float32)
    sgn = np.tile(np.array([-1.0, 1.0]), DH // 2)[:, None]
    sinT = (np.repeat(np.sin(ang), 2, axis=0) * sgn).astype(np.float32)

    permM = np.zeros((128, 128), dtype=np.float32)
    for j in range(0, 128, 2):
        permM[j + 1, j] = 1.0
        permM[j, j + 1] = 1.0

    p_i = np.arange(128)[:, None]
    f_i = np.arange(CH)[None, :]
    masks = np.concatenate(
        [(128 * dl + p_i <= f_i).astype(np.float32) for dl in range(nS)],
        axis=1).astype(BFNP)

    def tile_pmaj(w, ncols):
        return np.ascontiguousarray(
            np.asarray(w, dtype=np.float32).reshape(nA, 128, ncols)
            .transpose(1, 0, 2).reshape(128, nA * ncols)).astype(BFNP)

    in_maps = []
    for c in range(N_CORES):
        kv = c // 2
        wo_c = np.asarray(WO, dtype=np.float32)[256 * c:256 * (c + 1), :]
        in_maps.append({
            "xT": xTc,
            "wq2": tile_pmaj(np.asarray(WQ)[:, 256 * c:256 * (c + 1)], 256),
            "wk2": tile_pmaj(np.asarray(WK)[:, 128 * kv:128 * (kv + 1)], 128),
            "wv2": tile_pmaj(np.asarray(WV)[:, 128 * kv:128 * (kv + 1)], 128),
            "wo2": np.ascontiguousarray(
                wo_c.reshape(2, 128, D).transpose(1, 0, 2)
                .reshape(128, 2 * D)).astype(BFNP),
            "cosT": cosT, "sinT": sinT, "permM": permM, "masks": masks,
            "onescol": np.ones((128, 1), dtype=BFNP),
            "onesrow": np.ones((1, 128), dtype=np.float32),
        })
    return in_maps


_NC_CACHE = {}


def _get_nc():
    if "nc" not in _NC_CACHE:
        _NC_CACHE["nc"] = build_nc()
    return _NC_CACHE["nc"]


def run_on_hw(inputs, trace=False):
    """Returns (out [1,T,D] fp32, BassKernelResults)."""
    nc = _get_nc()
    in_maps = host_prep(inputs["x"], inputs["WQ"], inputs["WK"],
                        inputs["WV"], inputs["WO"])
    res = run_bass_kernel_spmd(nc, in_maps, list(range(N_CORES)),
                               trace=trace)
    acc = np.zeros((T, D), dtype=np.float64)
    for c in range(N_CORES):
        acc += res.results[c]["out"].astype(np.float64)
    return acc.astype(np.float32)[None], res


def kernel(x, WQ, WK, WV, WO):
    out, _ = run_on_hw({"x": x, "WQ": WQ, "WK": WK, "WV": WV, "WO": WO})
    return out



# revision 2
# speedup vs baseline: 1.0983x; 1.0983x over previous
"""GroupedQueryAttention (B=1, T=4096, D=2048, 16 q-heads / 4 kv-heads, RoPE,
causal) on 8 Trainium2 NeuronCores.

Sharding: head tensor-parallel, core c owns q-heads {2c, 2c+1} and kv head
c//2. Each core computes partial out = ctx_heads @ WO_slice over the full
sequence; bf16 partials are summed on the host.

Structure (vs the 682 us baseline; measured 514 us):
- attention in key-tile PAIRS: two S matmuls fill halves of a [128,1024]
  PSUM tile (2 banks) and ONE exp activation covers both, halving the
  scalar-engine instruction count (attention was ACT-bound).
- both heads' softmax denominators accumulate in one PSUM bank at
  partitions 0/32 (ones-column matmuls, PE col-group packed); a single
  wide reciprocal + bf16 ones-row broadcast matmuls normalize ctx.
- all matmul operands bf16 (fp32 PSUM accumulation); fp32 matmuls gone.
- V produced as long-stream V^T accumulation + PE transposes to natural
  [tokens, dh] (fewer LDWEIGHTS than 64 short matmuls per chunk).
- PSUM->SBUF copies split: q/k/v on scalar (idle in the proj window),
  WO output staging on vector; output written bf16 (halves out-DMA).
- software-pipelined emission: WO tiles of the previous chunk are
  interleaved into the ACT-bound attention loop as PE fillers (plus a
  pre-loop batch covering the RoPE DVE latency); normalization part 2 is
  deferred past the next chunk's projections so the PE never waits on the
  reciprocal chain.
- PSUM: one shared 4-bank pool (proj acc / S-pairs / rope perm / bcast)
  + 2 ctx banks + 1 denominator bank + 1 WO bank = 8 banks.
"""

import math

import numpy as np
import ml_dtypes

import concourse.bass as bass
import concourse.mybir as mybir
import concourse.tile as tile
from concourse.bass_utils import run_bass_kernel_spmd

FP = mybir.dt.float32
BF = mybir.dt.bfloat16
BFNP = ml_dtypes.bfloat16

T, D, DH = 4096, 2048, 128      # seq len, model dim, head dim
CH = 512                        # query-chunk (free dim of attention matmuls)
N_CORES = 8


# --------------------------------------------------------------------------
# workaround: this walrus build rejects instructions carrying >1 sem-waits
# (setupSyncWait "Too many sync wait commands"); split extras into NoOps.
_WS_CTR = [0]


def _split_multi_waits(nc, limit=1):
    for f in nc.m.functions:
        for bb in f.blocks:
            il = bb.instructions
            i = 0
            while i < len(il):
                inst = il[i]
                si = getattr(inst, "sync_info", None)
                if si is not None and len(si.on_wait) > limit:
                    waits = list(si.on_wait)
                    keep, rest = waits[:limit], waits[limit:]
                    nops = []
                    for j in range(0, len(rest), limit):
                        _WS_CTR[0] += 1
                        n = mybir.InstNoOp(name=f"waitsplit-{_WS_CTR[0]}")
                        n.engine = inst.engine
                        n.sync_info = mybir.SyncInfo(
                            on_wait=rest[j:j + limit], on_update=[])
                        nops.append(n)
                    inst.sync_info = mybir.SyncInfo(
                        on_wait=keep, on_update=list(si.on_update))
                    for k, n in enumerate(nops):
                        il.insert(i + k, n)
                    i += len(nops)
                i += 1


# --------------------------------------------------------------------------
def build_nc():
    nT = T // CH          # 8 T-chunks
    nA = D // 128         # 16 contraction tiles
    nS = CH // 128        # 4 kt subtiles per chunk
    nP = nS // 2          # 2 kt PAIRS per chunk
    nN = D // 512         # 4 output column tiles
    ISQ = 1.0 / math.sqrt(float(DH))

    nc = bass.Bass()

    xT = nc.dram_tensor("xT", [D, T], BF, kind="ExternalInput")
    wq2 = nc.dram_tensor("wq2", [128, nA * 256], BF, kind="ExternalInput")
    wk2 = nc.dram_tensor("wk2", [128, nA * 128], BF, kind="ExternalInput")
    wv2 = nc.dram_tensor("wv2", [128, nA * 128], BF, kind="ExternalInput")
    wo2 = nc.dram_tensor("wo2", [128, 2 * D], BF, kind="ExternalInput")
    cosT = nc.dram_tensor("cosT", [128, T], BF, kind="ExternalInput")
    sinT = nc.dram_tensor("sinT", [128, T], BF, kind="ExternalInput")
    permM = nc.dram_tensor("permM", [128, 128], BF, kind="ExternalInput")
    masks = nc.dram_tensor("masks", [128, nS * CH], BF, kind="ExternalInput")
    onescol = nc.dram_tensor("onescol", [128, 1], BF, kind="ExternalInput")
    onesrow = nc.dram_tensor("onesrow", [64, 128], BF, kind="ExternalInput")
    ident = nc.dram_tensor("ident", [128, 128], BF, kind="ExternalInput")
    out = nc.dram_tensor("out", [T, D], BF, kind="ExternalOutput")

    xTr = xT.rearrange("(a p) t -> p a t", p=128)

    with tile.TileContext(nc) as tc:
        with (
            tc.tile_pool(name="res", bufs=1) as res,
            tc.tile_pool(name="ktv", bufs=2 * nT) as ktv,
            tc.tile_pool(name="xt", bufs=32) as xtp,
            tc.tile_pool(name="qk", bufs=2) as qkp,     # q/k sbuf copies
            tc.tile_pool(name="qr", bufs=4) as qrp,     # rope outputs q heads
            tc.tile_pool(name="rt", bufs=4) as rtp,     # rope temporaries
            tc.tile_pool(name="pp", bufs=3) as ppp,     # P pair tiles
            tc.tile_pool(name="nrm", bufs=2) as nrm,    # recip / bcast / cn
            tc.tile_pool(name="oo", bufs=4) as oop,     # out staging bf16
            tc.tile_pool(name="psX", bufs=2, space="PSUM") as psX,
            tc.tile_pool(name="psC", bufs=2, space="PSUM") as psC,
            tc.tile_pool(name="psL", bufs=1, space="PSUM") as psL,
            tc.tile_pool(name="psW", bufs=1, space="PSUM") as psW,
        ):
            wq_sb = res.tile([128, nA * 256], BF, name="wq_sb")
            wk_sb = res.tile([128, nA * 128], BF, name="wk_sb")
            wv_sb = res.tile([128, nA * 128], BF, name="wv_sb")
            wo_sb = res.tile([128, 2 * D], BF, name="wo_sb")
            mask_sb = res.tile([128, nS * CH], BF, name="mask_sb")
            perm_sb = res.tile([128, 128], BF, name="perm_sb")
            oc_sb = res.tile([128, 1], BF, name="oc_sb")
            or_sb = res.tile([64, 128], BF, name="or_sb")
            id_sb = res.tile([128, 128], BF, name="id_sb")
            cos_sb = res.tile([128, T], BF, name="cos_sb")
            sin_sb = res.tile([128, T], BF, name="sin_sb")
            for t_, s_ in [
                (wq_sb, wq2), (wk_sb, wk2), (wv_sb, wv2), (wo_sb, wo2),
                (mask_sb, masks), (perm_sb, permM), (oc_sb, onescol),
                (or_sb, onesrow), (id_sb, ident), (cos_sb, cosT),
                (sin_sb, sinT),
            ]:
                nc.sync.dma_start(t_[:], s_[:])

            kt_tiles = []
            v_tiles = []
            x_chunks = {}

            def load_x(qc):
                t0 = qc * CH
                xts = []
                for a in range(nA):
                    xa = xtp.tile([128, CH], BF, name=f"x{qc}_{a}", tag="xt")
                    nc.sync.dma_start(xa[:], xTr[:, a, t0:t0 + CH])
                    xts.append(xa)
                x_chunks[qc] = xts

            load_x(0)

            def emit_proj_rope(qc):
                """Q/K/V projections + RoPE for chunk qc. Returns (qr0, qr1)."""
                t0 = qc * CH
                xts = x_chunks[qc]

                def accum(lhs, N, nm):
                    acc = psX.tile([128, N], FP, name=f"ps_{nm}", tag="m",
                                   padded_shape=[128, 1024])
                    for a in range(nA):
                        nc.tensor.matmul(
                            acc[:], lhs(a), xts[a][:],
                            start=(a == 0), stop=(a == nA - 1),
                        )
                    return acc

                # Q0 / Q1 / K projections -> SBUF bf16 copies
                # (gpsimd cannot read PSUM; use vector)
                srcs = []
                for h, nm in ((0, "q0"), (1, "q1")):
                    ps = accum(lambda a, h=h: wq_sb[:, a * 256 + h * 128:
                                                    a * 256 + h * 128 + 128],
                               CH, f"{nm}_{qc}")
                    sb_ = qkp.tile([128, CH], BF, name=f"{nm}s{qc}", tag="qk",
                                   bufs=4)
                    nc.scalar.copy(sb_[:], ps[:])
                    srcs.append(sb_)
                ps = accum(lambda a: wk_sb[:, a * 128:(a + 1) * 128],
                           CH, f"k_{qc}")
                k_sb = qkp.tile([128, CH], BF, name=f"ks{qc}", tag="qk",
                                bufs=4)
                nc.scalar.copy(k_sb[:], ps[:])
                srcs.append(k_sb)

                # V: long-stream V^T accum (weights stationary), then PE
                # transposes to natural [tokens, dh] — far fewer LDWEIGHTS
                # than 64 short matmuls with x-slices as weights.
                vt_ps = accum(lambda a: wv_sb[:, a * 128:(a + 1) * 128],
                              CH, f"vT_{qc}")
                vT_sb = qkp.tile([128, CH], BF, name=f"vT{qc}", tag="qk",
                                 bufs=4)
                nc.scalar.copy(vT_sb[:], vt_ps[:])
                vt = ktv.tile([128, nS * 128], BF, name=f"v{qc}", tag="ktv")
                for s in range(nS):
                    tp = psX.tile([128, 128], BF, name=f"tp{qc}_{s}",
                                  tag="m", padded_shape=[128, 1024])
                    nc.tensor.transpose(tp[:],
                                        vT_sb[:, s * 128:(s + 1) * 128],
                                        id_sb[:])
                    nc.vector.tensor_copy(vt[:, s * 128:(s + 1) * 128],
                                          tp[:])
                v_tiles.append(vt)

                # RoPE: dst = src*cos + perm(src)*sin_signed  (bf16)
                cos_t = cos_sb[:, t0:t0 + CH]
                sin_t = sin_sb[:, t0:t0 + CH]
                qr0 = qrp.tile([128, CH], BF, name=f"qr0_{qc}", tag="qr")
                qr1 = qrp.tile([128, CH], BF, name=f"qr1_{qc}", tag="qr")
                ktt = ktv.tile([128, CH], BF, name=f"kt{qc}", tag="ktv")
                for src_sb, dst in ((srcs[0], qr0), (srcs[1], qr1),
                                    (srcs[2], ktt)):
                    sw = psX.tile([128, CH], FP, name=f"sw_{qc}", tag="m",
                                  padded_shape=[128, 1024])
                    nc.tensor.matmul(sw[:], perm_sb[:], src_sb[:])
                    t1 = rtp.tile([128, CH], BF, name=f"r1_{qc}", tag="rt")
                    nc.vector.tensor_mul(t1[:], src_sb[:], cos_t)
                    t2 = rtp.tile([128, CH], BF, name=f"r2_{qc}", tag="rt")
                    nc.vector.tensor_mul(t2[:], sw[:], sin_t)
                    nc.vector.tensor_add(dst[:], t1[:], t2[:])
                kt_tiles.append(ktt)
                return qr0, qr1

            def emit_attention(qc, qr0, qr1, fillers):
                """Causal attention for chunk qc; `fillers` are deferred PE
                emitters (previous chunk's WO tiles) interleaved into the
                ACT-bound pair loop to keep the tensor engine warm.
                Returns (cn0, cn1) bf16."""
                npair = (qc + 1) * nP
                qrs = (qr0, qr1)
                fillers = list(fillers)

                ctx = [psC.tile([128, CH], FP, name=f"ctx{qc}_{h}", tag="ctx")
                       for h in range(2)]
                # both heads' denominators in ONE bank: h0 -> partition 0,
                # h1 -> partition 32 (PE col-group packing runs them
                # concurrently; one reciprocal covers both)
                lt = psL.tile([128, CH], FP, name=f"l{qc}", tag="l")
                lps = [lt[0:1, :], lt[32:33, :]]

                def emit_spair(p, h):
                    kc, s0 = (2 * p) // nS, (2 * p) % nS
                    sp = psX.tile([128, 2 * CH], FP, name=f"S{qc}_{h}_{p}",
                                  tag="m")
                    kt = kt_tiles[kc]
                    nc.tensor.matmul(sp[:, 0:CH],
                                     kt[:, s0 * 128:(s0 + 1) * 128], qrs[h][:])
                    nc.tensor.matmul(sp[:, CH:2 * CH],
                                     kt[:, (s0 + 1) * 128:(s0 + 2) * 128],
                                     qrs[h][:])
                    return sp

                def emit_exp(p, h, sp):
                    pt = ppp.tile([128, 2 * CH], BF, name=f"P{qc}_{h}_{p}",
                                  tag="p")
                    nc.scalar.activation(pt[:], sp[:],
                                         mybir.ActivationFunctionType.Exp,
                                         scale=ISQ)
                    kc, s0 = (2 * p) // nS, (2 * p) % nS
                    if kc == qc:  # diagonal chunk: causal mask
                        nc.vector.tensor_mul(
                            pt[:], pt[:],
                            mask_sb[:, s0 * CH:(s0 + 2) * CH])
                    return pt

                def emit_lav(p, h, pt):
                    kc, s0 = (2 * p) // nS, (2 * p) % nS
                    vt = v_tiles[kc]
                    st, sp_ = (p == 0), (p == npair - 1)
                    nc.tensor.matmul(lps[h], oc_sb[:], pt[:, 0:CH],
                                     start=st, stop=False)
                    nc.tensor.matmul(lps[h], oc_sb[:], pt[:, CH:2 * CH],
                                     start=False, stop=sp_)
                    nc.tensor.matmul(ctx[h][:],
                                     vt[:, s0 * 128:(s0 + 1) * 128],
                                     pt[:, 0:CH], start=st, stop=False)
                    nc.tensor.matmul(ctx[h][:],
                                     vt[:, (s0 + 1) * 128:(s0 + 2) * 128],
                                     pt[:, CH:2 * CH], start=False, stop=sp_)

                # pre-loop fillers cover the PE wait on the RoPE DVE chain
                for _ in range(4):
                    if fillers:
                        fillers.pop(0)()
                # software pipeline: S(p) issued one step ahead of l/AV(p)
                sp0 = emit_spair(0, 0)
                sp1 = emit_spair(0, 1)
                pt0 = emit_exp(0, 0, sp0)
                pt1 = emit_exp(0, 1, sp1)
                for p in range(npair):
                    if p + 1 < npair:
                        spn0 = emit_spair(p + 1, 0)
                        spn1 = emit_spair(p + 1, 1)
                        ptn0 = emit_exp(p + 1, 0, spn0)
                        ptn1 = emit_exp(p + 1, 1, spn1)
                    emit_lav(p, 0, pt0)
                    emit_lav(p, 1, pt1)
                    for _ in range(2):   # keep PE fed during ACT overhang
                        if len(fillers) > 2:
                            fillers.pop(0)()
                    if p + 1 < npair:
                        pt0, pt1 = ptn0, ptn1
                for f in fillers:        # reserved: run during recip latency
                    f()

                # normalization part 1: one wide reciprocal covers both
                # heads' rows (rows 1..31, 33..63 are unwritten-PSUM
                # garbage, unused). Part 2 (broadcast + ctx*bc) is deferred
                # until after the next chunk's projections so the PE is not
                # stalled on this DVE chain.
                r_t = nrm.tile([64, CH], FP, name=f"r{qc}", tag="r")
                nc.vector.reciprocal(r_t[:], lt[0:64, :])
                rb_t = nrm.tile([64, CH], BF, name=f"rb{qc}", tag="rb")
                nc.vector.tensor_copy(rb_t[:], r_t[:])
                return rb_t, ctx

            def emit_norm2(qc, rb_t, ctx):
                """Normalization part 2: bf16 ones-row broadcast matmuls,
                then cn = ctx * (1/l)."""
                cns = []
                for h in range(2):
                    bc_ps = psX.tile([128, CH], FP, name=f"bc{qc}_{h}",
                                     tag="m", padded_shape=[128, 1024])
                    nc.tensor.matmul(bc_ps[:],
                                     or_sb[32 * h:32 * h + 1, :],
                                     rb_t[32 * h:32 * h + 1, :])
                    bc_sb = nrm.tile([128, CH], FP, name=f"bcs{qc}_{h}",
                                     tag="bc")
                    nc.scalar.copy(bc_sb[:], bc_ps[:])
                    cn = nrm.tile([128, CH], BF, name=f"cn{qc}_{h}", tag="cn")
                    nc.vector.tensor_mul(cn[:], ctx[h][:], bc_sb[:])
                    cns.append(cn)
                return cns

            def wo_fillers(qc, cns):
                """One emitter per WO tile: 2 matmuls -> direct PSUM->HBM DMA."""
                t0 = qc * CH

                def mk(s, n):
                    def emit():
                        w_ps = psW.tile([128, 512], FP,
                                        name=f"w{qc}_{s}_{n}", tag="w")
                        nc.tensor.matmul(
                            w_ps[:], cns[0][:, s * 128:(s + 1) * 128],
                            wo_sb[:, n * 512:(n + 1) * 512],
                            start=True, stop=False)
                        nc.tensor.matmul(
                            w_ps[:], cns[1][:, s * 128:(s + 1) * 128],
                            wo_sb[:, D + n * 512:D + (n + 1) * 512],
                            start=False, stop=True)
                        osb = oop.tile([128, 512], BF, name=f"o{qc}_{s}_{n}",
                                       tag="osb")
                        nc.vector.tensor_copy(osb[:], w_ps[:])
                        nc.sync.dma_start(
                            out[t0 + s * 128:t0 + (s + 1) * 128,
                                n * 512:(n + 1) * 512], osb[:])
                    return emit
                return [mk(s, n) for s in range(nS) for n in range(nN)]

            # ---- main schedule ----
            fillers = []       # pending WO tile emitters
            pend = None        # (qc, rb_t, ctx) awaiting norm part 2
            for qc in range(nT):
                qr0, qr1 = emit_proj_rope(qc)
                if qc + 1 < nT:
                    load_x(qc + 1)
                if pend is not None:
                    cns = emit_norm2(pend[0], pend[1], pend[2])
                    fillers = wo_fillers(pend[0], cns)
                rb_t, ctx = emit_attention(qc, qr0, qr1, fillers)
                pend = (qc, rb_t, ctx)
            cns = emit_norm2(pend[0], pend[1], pend[2])
            for f in wo_fillers(pend[0], cns):
                f()

    _split_multi_waits(nc, 1)
    return nc


# --------------------------------------------------------------------------
def host_prep(x, WQ, WK, WV, WO):
    nA = D // 128
    nS = CH // 128
    ROPE_BASE = 10000.0

    xTc = np.ascontiguousarray(
        np.asarray(x, dtype=np.float32).reshape(T, D).T).astype(BFNP)

    omega = 1.0 / (ROPE_BASE ** (np.arange(0, DH, 2, dtype=np.float64) / DH))
    ang = np.outer(omega, np.arange(T, dtype=np.float64))
    cosT = np.repeat(np.cos(ang), 2, axis=0).astype(BFNP)
    sgn = np.tile(np.array([-1.0, 1.0]), DH // 2)[:, None]
    sinT = (np.repeat(np.sin(ang), 2, axis=0) * sgn).astype(BFNP)

    permM = np.zeros((128, 128), dtype=np.float32)
    for j in range(0, 128, 2):
        permM[j + 1, j] = 1.0
        permM[j, j + 1] = 1.0
    permM = permM.astype(BFNP)

    p_i = np.arange(128)[:, None]
    f_i = np.arange(CH)[None, :]
    masks = np.concatenate(
        [(128 * dl + p_i <= f_i).astype(np.float32) for dl in range(nS)],
        axis=1).astype(BFNP)

    def tile_pmaj(w, ncols):
        return np.ascontiguousarray(
            np.asarray(w, dtype=np.float32).reshape(nA, 128, ncols)
            .transpose(1, 0, 2).reshape(128, nA * ncols)).astype(BFNP)

    in_maps = []
    for c in range(N_CORES):
        kv = c // 2
        wo_c = np.asarray(WO, dtype=np.float32)[256 * c:256 * (c + 1), :]
        in_maps.append({
            "xT": xTc,
            "wq2": tile_pmaj(np.asarray(WQ)[:, 256 * c:256 * (c + 1)], 256),
            "wk2": tile_pmaj(np.asarray(WK)[:, 128 * kv:128 * (kv + 1)], 128),
            "wv2": tile_pmaj(np.asarray(WV)[:, 128 * kv:128 * (kv + 1)], 128),
            "wo2": np.ascontiguousarray(
                wo_c.reshape(2, 128, D).transpose(1, 0, 2)
                .reshape(128, 2 * D)).astype(BFNP),
            "cosT": cosT, "sinT": sinT, "permM": permM, "masks": masks,
            "onescol": np.ones((128, 1), dtype=BFNP),
            "onesrow": np.ones((64, 128), dtype=BFNP),
            "ident": np.eye(128, dtype=np.float32).astype(BFNP),
        })
    return in_maps


_NC_CACHE = {}


def _get_nc():
    if "nc" not in _NC_CACHE:
        _NC_CACHE["nc"] = build_nc()
    return _NC_CACHE["nc"]


def run_on_hw(inputs, trace=False):
    """Returns (out [1,T,D] fp32, BassKernelResults)."""
    nc = _get_nc()
    in_maps = host_prep(inputs["x"], inputs["WQ"], inputs["WK"],
                        inputs["WV"], inputs["WO"])
    res = run_bass_kernel_spmd(nc, in_maps, list(range(N_CORES)),
                               trace=trace)
    acc = np.zeros((T, D), dtype=np.float64)
    for c in range(N_CORES):
        acc += res.results[c]["out"].astype(np.float64)
    return acc.astype(np.float32)[None], res


def kernel(x, WQ, WK, WV, WO):
    out, _ = run_on_hw({"x": x, "WQ": WQ, "WK": WK, "WV": WV, "WO": WO})
    return out


# revision 6
# speedup vs baseline: 1.1152x; 1.0154x over previous
"""GroupedQueryAttention (B=1, T=4096, D=2048, 16 q-heads / 4 kv-heads, RoPE,
causal) on 8 Trainium2 NeuronCores — v6.

Sharding: head tensor-parallel, core c owns q-heads {2c, 2c+1} and kv head
c//2. Each core computes partial out = ctx_heads @ WO_slice over the full
sequence; bf16 partials are summed on the host.

v6 idea: ONE dense tensor-engine stream. All chunk-boundary PE work
(projection accums, RoPE perm, V transposes, normalization broadcast, WO
output tiles) is queued as filler closures and drained inside the
ACT-paced attention loops, so the PE never ping-pongs between a PE-only
projection window and an ACT-bound attention window (which left HAM
oscillating at low clock).

PSUM: psX 3x[128,512] (S tiles, attention-only) + psC 2 (ctx per head)
+ psL 1 (both heads' denominators at partitions 0/32, col-packed ones
matmuls) + psW 2 (all transient boundary tiles) = 8 banks.
"""

import math

import numpy as np
import ml_dtypes

import concourse.bass as bass
import concourse.mybir as mybir
import concourse.tile as tile
from concourse.bass_utils import run_bass_kernel_spmd

FP = mybir.dt.float32
BF = mybir.dt.bfloat16
BFNP = ml_dtypes.bfloat16

T, D, DH = 4096, 2048, 128      # seq len, model dim, head dim
CH = 512                        # query-chunk (free dim of attention matmuls)
N_CORES = 8


# --------------------------------------------------------------------------
# workaround: this walrus build rejects instructions carrying >1 sem-waits
# (setupSyncWait "Too many sync wait commands"); split extras into NoOps.
_WS_CTR = [0]


def _split_multi_waits(nc, limit=1):
    for f in nc.m.functions:
        for bb in f.blocks:
            il = bb.instructions
            i = 0
            while i < len(il):
                inst = il[i]
                si = getattr(inst, "sync_info", None)
                if si is not None and len(si.on_wait) > limit:
                    waits = list(si.on_wait)
                    keep, rest = waits[:limit], waits[limit:]
                    nops = []
                    for j in range(0, len(rest), limit):
                        _WS_CTR[0] += 1
                        n = mybir.InstNoOp(name=f"waitsplit-{_WS_CTR[0]}")
                        n.engine = inst.engine
                        n.sync_info = mybir.SyncInfo(
                            on_wait=rest[j:j + limit], on_update=[])
                        nops.append(n)
                    inst.sync_info = mybir.SyncInfo(
                        on_wait=keep, on_update=list(si.on_update))
                    for k, n in enumerate(nops):
                        il.insert(i + k, n)
                    i += len(nops)
                i += 1


# --------------------------------------------------------------------------
def build_nc():
    nT = T // CH          # 8 T-chunks
    nA = D // 128         # 16 contraction tiles
    nS = CH // 128        # 4 kt subtiles per chunk
    nN = D // 512         # 4 output column tiles
    ISQ = 1.0 / math.sqrt(float(DH))

    nc = bass.Bass()

    xT = nc.dram_tensor("xT", [D, T], BF, kind="ExternalInput")
    wq2 = nc.dram_tensor("wq2", [128, nA * 256], BF, kind="ExternalInput")
    wk2 = nc.dram_tensor("wk2", [128, nA * 128], BF, kind="ExternalInput")
    wv2 = nc.dram_tensor("wv2", [128, nA * 128], BF, kind="ExternalInput")
    wo2 = nc.dram_tensor("wo2", [128, 2 * D], BF, kind="ExternalInput")
    cosT = nc.dram_tensor("cosT", [128, T], BF, kind="ExternalInput")
    sinT = nc.dram_tensor("sinT", [128, T], BF, kind="ExternalInput")
    permM = nc.dram_tensor("permM", [128, 128], BF, kind="ExternalInput")
    masks = nc.dram_tensor("masks", [128, nS * CH], BF, kind="ExternalInput")
    onescol = nc.dram_tensor("onescol", [128, 1], BF, kind="ExternalInput")
    onesrow = nc.dram_tensor("onesrow", [64, 128], BF, kind="ExternalInput")
    ident = nc.dram_tensor("ident", [128, 128], BF, kind="ExternalInput")
    out = nc.dram_tensor("out", [T, D], BF, kind="ExternalOutput")

    xTr = xT.rearrange("(a p) t -> p a t", p=128)

    with tile.TileContext(nc) as tc:
        with (
            tc.tile_pool(name="res", bufs=1) as res,
            tc.tile_pool(name="ktv", bufs=2 * nT) as ktv,
            tc.tile_pool(name="xt", bufs=32) as xtp,
            tc.tile_pool(name="qk", bufs=4) as qkp,     # q/k/vT sbuf copies
            tc.tile_pool(name="qr", bufs=4) as qrp,     # rope outputs q heads
            tc.tile_pool(name="rt", bufs=4) as rtp,     # rope temporaries
            tc.tile_pool(name="pp", bufs=4) as ppp,     # P tiles
            tc.tile_pool(name="nrm", bufs=2) as nrm,    # recip / bcast / cn
            tc.tile_pool(name="oo", bufs=4) as oop,     # out staging bf16
            tc.tile_pool(name="psX", bufs=3, space="PSUM") as psX,
            tc.tile_pool(name="psC", bufs=2, space="PSUM") as psC,
            tc.tile_pool(name="psL", bufs=1, space="PSUM") as psL,
            tc.tile_pool(name="psW", bufs=2, space="PSUM") as psW,
        ):
            wq_sb = res.tile([128, nA * 256], BF, name="wq_sb")
            wk_sb = res.tile([128, nA * 128], BF, name="wk_sb")
            wv_sb = res.tile([128, nA * 128], BF, name="wv_sb")
            wo_sb = res.tile([128, 2 * D], BF, name="wo_sb")
            mask_sb = res.tile([128, nS * CH], BF, name="mask_sb")
            perm_sb = res.tile([128, 128], BF, name="perm_sb")
            oc_sb = res.tile([128, 1], BF, name="oc_sb")
            or_sb = res.tile([64, 128], BF, name="or_sb")
            id_sb = res.tile([128, 128], BF, name="id_sb")
            cos_sb = res.tile([128, T], BF, name="cos_sb")
            sin_sb = res.tile([128, T], BF, name="sin_sb")

            kt_tiles = {}
            v_tiles = {}
            x_chunks = {}
            qr_chunks = {}

            def load_x(qc):
                t0 = qc * CH
                xts = []
                for a in range(nA):
                    xa = xtp.tile([128, CH], BF, name=f"x{qc}_{a}", tag="xt")
                    nc.sync.dma_start(xa[:], xTr[:, a, t0:t0 + CH])
                    xts.append(xa)
                x_chunks[qc] = xts

            # x for chunk 0 first so projections can start ASAP, then wq,
            # then everything else.
            load_x(0)
            for t_, s_ in [
                (wq_sb, wq2), (wk_sb, wk2), (wv_sb, wv2), (cos_sb, cosT),
                (sin_sb, sinT), (perm_sb, permM), (mask_sb, masks),
                (oc_sb, onescol), (or_sb, onesrow), (id_sb, ident),
                (wo_sb, wo2),
            ]:
                nc.sync.dma_start(t_[:], s_[:])

            # ---------- projection / rope closures ----------
            def proj_closures(qc):
                """Filler closures, in dependency order, computing Q/K/V^T
                projections, V transposes, and RoPE for chunk qc."""
                t0 = qc * CH
                cos_t = cos_sb[:, t0:t0 + CH]
                sin_t = sin_sb[:, t0:t0 + CH]
                qr0 = qrp.tile([128, CH], BF, name=f"qr0_{qc}", tag="qr")
                qr1 = qrp.tile([128, CH], BF, name=f"qr1_{qc}", tag="qr")
                ktt = ktv.tile([128, CH], BF, name=f"kt{qc}", tag="ktv")
                qr_chunks[qc] = (qr0, qr1)
                kt_tiles[qc] = ktt
                vt = ktv.tile([128, nS * 128], BF, name=f"v{qc}", tag="ktv")
                v_tiles[qc] = vt

                sb_holder = {}

                def mk_accum(lhs, nm):
                    def emit():
                        xts = x_chunks[qc]
                        acc = psW.tile([128, CH], FP, name=f"ps_{nm}",
                                       tag="w")
                        for a in range(nA):
                            nc.tensor.matmul(
                                acc[:], lhs(a), xts[a][:],
                                start=(a == 0), stop=(a == nA - 1))
                        sb_ = qkp.tile([128, CH], BF, name=f"{nm}s",
                                       tag="qk")
                        nc.scalar.copy(sb_[:], acc[:])
                        sb_holder[nm] = sb_
                    return emit

                def mk_rope(nm, dst):
                    def emit():
                        src_sb = sb_holder[nm]
                        sw = psW.tile([128, CH], FP, name=f"sw_{nm}",
                                      tag="w")
                        nc.tensor.matmul(sw[:], perm_sb[:], src_sb[:])
                        t1 = rtp.tile([128, CH], BF, name=f"r1_{nm}",
                                      tag="rt")
                        nc.vector.tensor_mul(t1[:], src_sb[:], cos_t)
                        t2 = rtp.tile([128, CH], BF, name=f"r2_{nm}",
                                      tag="rt")
                        nc.vector.tensor_mul(t2[:], sw[:], sin_t)
                        nc.vector.tensor_add(dst[:], t1[:], t2[:])
                    return emit

                def mk_vtrans():
                    def emit():
                        vT_sb = sb_holder[f"vT{qc}"]
                        for s in range(nS):
                            tp = psW.tile([128, 128], BF, name=f"tp{qc}_{s}",
                                          tag="w", padded_shape=[128, 512])
                            nc.tensor.transpose(
                                tp[:], vT_sb[:, s * 128:(s + 1) * 128],
                                id_sb[:])
                            nc.vector.tensor_copy(
                                vt[:, s * 128:(s + 1) * 128], tp[:])
                    return emit

                cl = []
                cl.append(mk_accum(lambda a: wq_sb[:, a * 256:a * 256 + 128],
                                   f"q0{qc}"))
                cl.append(mk_rope(f"q0{qc}", qr0))
                cl.append(mk_accum(lambda a: wq_sb[:, a * 256 + 128:
                                                   a * 256 + 256],
                                   f"q1{qc}"))
                cl.append(mk_rope(f"q1{qc}", qr1))
                cl.append(mk_accum(lambda a: wk_sb[:, a * 128:(a + 1) * 128],
                                   f"k{qc}"))
                cl.append(mk_rope(f"k{qc}", ktt))
                cl.append(mk_accum(lambda a: wv_sb[:, a * 128:(a + 1) * 128],
                                   f"vT{qc}"))
                cl.append(mk_vtrans())
                return cl

            # ---------- normalization part 2 + WO closures ----------
            def norm2_wo_closures(qc, rb_t, ctx):
                t0 = qc * CH
                cns = []

                def mk_norm(h):
                    def emit():
                        bc_ps = psW.tile([128, CH], FP, name=f"bc{qc}_{h}",
                                         tag="w")
                        nc.tensor.matmul(bc_ps[:],
                                         or_sb[32 * h:32 * h + 1, :],
                                         rb_t[32 * h:32 * h + 1, :])
                        bc_sb = nrm.tile([128, CH], FP, name=f"bcs{qc}_{h}",
                                         tag="bc")
                        nc.scalar.copy(bc_sb[:], bc_ps[:])
                        cn = nrm.tile([128, CH], BF, name=f"cn{qc}_{h}",
                                      tag="cn")
                        nc.vector.tensor_mul(cn[:], ctx[h][:], bc_sb[:])
                        cns.append(cn)
                    return emit

                def mk_wo(s, n):
                    def emit():
                        w_ps = psW.tile([128, 512], FP,
                                        name=f"w{qc}_{s}_{n}", tag="w")
                        nc.tensor.matmul(
                            w_ps[:], cns[0][:, s * 128:(s + 1) * 128],
                            wo_sb[:, n * 512:(n + 1) * 512],
                            start=True, stop=False)
                        nc.tensor.matmul(
                            w_ps[:], cns[1][:, s * 128:(s + 1) * 128],
                            wo_sb[:, D + n * 512:D + (n + 1) * 512],
                            start=False, stop=True)
                        osb = oop.tile([128, 512], BF, name=f"o{qc}_{s}_{n}",
                                       tag="osb")
                        nc.vector.tensor_copy(osb[:], w_ps[:])
                        nc.sync.dma_start(
                            out[t0 + s * 128:t0 + (s + 1) * 128,
                                n * 512:(n + 1) * 512], osb[:])
                    return emit

                return ([mk_norm(0), mk_norm(1)] +
                        [mk_wo(s, n) for s in range(nS) for n in range(nN)])

            # ---------- attention ----------
            def emit_attention(qc, queue):
                """Causal attention for chunk qc; pops filler closures from
                `queue` inside the loop and drains it at the end."""
                nkt = (qc + 1) * nS
                qr0, qr1 = qr_chunks[qc]
                qrs = (qr0, qr1)

                ctx = [psC.tile([128, CH], FP, name=f"ctx{qc}_{h}", tag="ctx")
                       for h in range(2)]
                lt = psL.tile([128, CH], FP, name=f"l{qc}", tag="l")
                lps = [lt[0:1, :], lt[32:33, :]]

                def emit_s(kt, h):
                    kc, ko = kt // nS, (kt % nS) * 128
                    sp = psX.tile([128, CH], FP, name=f"S{qc}_{h}_{kt}",
                                  tag="s")
                    nc.tensor.matmul(sp[:], kt_tiles[kc][:, ko:ko + 128],
                                     qrs[h][:])
                    return sp

                def emit_exp(kt, h, sp):
                    pt = ppp.tile([128, CH], BF, name=f"P{qc}_{h}_{kt}",
                                  tag="p")
                    nc.scalar.activation(pt[:], sp[:],
                                         mybir.ActivationFunctionType.Exp,
                                         scale=ISQ)
                    delta = kt - qc * nS
                    if delta >= 0:  # diagonal chunk: causal mask
                        nc.vector.tensor_mul(
                            pt[:], pt[:],
                            mask_sb[:, delta * CH:(delta + 1) * CH])
                    return pt

                # pipeline: S/exp one step ahead; l pair adjacent (packed)
                p0 = emit_exp(0, 0, emit_s(0, 0))
                p1 = emit_exp(0, 1, emit_s(0, 1))
                for kt in range(nkt):
                    if kt + 1 < nkt:
                        pn0 = emit_exp(kt + 1, 0, emit_s(kt + 1, 0))
                        pn1 = emit_exp(kt + 1, 1, emit_s(kt + 1, 1))
                    st, sp_ = (kt == 0), (kt == nkt - 1)
                    nc.tensor.matmul(lps[0], oc_sb[:], p0[:],
                                     start=st, stop=sp_)
                    nc.tensor.matmul(lps[1], oc_sb[:], p1[:],
                                     start=st, stop=sp_)
                    kc, ko = kt // nS, (kt % nS) * 128
                    vt = v_tiles[kc]
                    nc.tensor.matmul(ctx[0][:], vt[:, ko:ko + 128], p0[:],
                                     start=st, stop=sp_)
                    nc.tensor.matmul(ctx[1][:], vt[:, ko:ko + 128], p1[:],
                                     start=st, stop=sp_)
                    if queue:
                        queue.pop(0)()
                    if kt + 1 < nkt:
                        p0, p1 = pn0, pn1
                while queue:   # must drain: next chunk needs qr/kt ready
                    queue.pop(0)()

                # normalization part 1 (DVE): one wide reciprocal, bf16 cast
                r_t = nrm.tile([64, CH], FP, name=f"r{qc}", tag="r")
                nc.vector.reciprocal(r_t[:], lt[0:64, :])
                rb_t = nrm.tile([64, CH], BF, name=f"rb{qc}", tag="rb")
                nc.vector.tensor_copy(rb_t[:], r_t[:])
                return rb_t, ctx

            # ---------- main schedule ----------
            queue = []
            for cl in proj_closures(0):
                cl()
            for qc in range(nT):
                if qc + 1 < nT:
                    load_x(qc + 1)
                    queue.extend(proj_closures(qc + 1))
                rb_t, ctx = emit_attention(qc, queue)
                queue.extend(norm2_wo_closures(qc, rb_t, ctx))
            for cl in queue:
                cl()

    _split_multi_waits(nc, 1)
    return nc


# --------------------------------------------------------------------------
def host_prep(x, WQ, WK, WV, WO):
    nA = D // 128
    nS = CH // 128
    ROPE_BASE = 10000.0

    xTc = np.ascontiguousarray(
        np.asarray(x, dtype=np.float32).reshape(T, D).T).astype(BFNP)

    omega = 1.0 / (ROPE_BASE ** (np.arange(0, DH, 2, dtype=np.float64) / DH))
    ang = np.outer(omega, np.arange(T, dtype=np.float64))
    cosT = np.repeat(np.cos(ang), 2, axis=0).astype(BFNP)
    sgn = np.tile(np.array([-1.0, 1.0]), DH // 2)[:, None]
    sinT = (np.repeat(np.sin(ang), 2, axis=0) * sgn).astype(BFNP)

    permM = np.zeros((128, 128), dtype=np.float32)
    for j in range(0, 128, 2):
        permM[j + 1, j] = 1.0
        permM[j, j + 1] = 1.0
    permM = permM.astype(BFNP)

    p_i = np.arange(128)[:, None]
    f_i = np.arange(CH)[None, :]
    masks = np.concatenate(
        [(128 * dl + p_i <= f_i).astype(np.float32) for dl in range(nS)],
        axis=1).astype(BFNP)

    def tile_pmaj(w, ncols):
        return np.ascontiguousarray(
            np.asarray(w, dtype=np.float32).reshape(nA, 128, ncols)
            .transpose(1, 0, 2).reshape(128, nA * ncols)).astype(BFNP)

    in_maps = []
    for c in range(N_CORES):
        kv = c // 2
        wo_c = np.asarray(WO, dtype=np.float32)[256 * c:256 * (c + 1), :]
        in_maps.append({
            "xT": xTc,
            "wq2": tile_pmaj(np.asarray(WQ)[:, 256 * c:256 * (c + 1)], 256),
            "wk2": tile_pmaj(np.asarray(WK)[:, 128 * kv:128 * (kv + 1)], 128),
            "wv2": tile_pmaj(np.asarray(WV)[:, 128 * kv:128 * (kv + 1)], 128),
            "wo2": np.ascontiguousarray(
                wo_c.reshape(2, 128, D).transpose(1, 0, 2)
                .reshape(128, 2 * D)).astype(BFNP),
            "cosT": cosT, "sinT": sinT, "permM": permM, "masks": masks,
            "onescol": np.ones((128, 1), dtype=BFNP),
            "onesrow": np.ones((64, 128), dtype=BFNP),
            "ident": np.eye(128, dtype=np.float32).astype(BFNP),
        })
    return in_maps


_NC_CACHE = {}


def _get_nc():
    if "nc" not in _NC_CACHE:
        _NC_CACHE["nc"] = build_nc()
    return _NC_CACHE["nc"]


def run_on_hw(inputs, trace=False):
    """Returns (out [1,T,D] fp32, BassKernelResults)."""
    nc = _get_nc()
    in_maps = host_prep(inputs["x"], inputs["WQ"], inputs["WK"],
                        inputs["WV"], inputs["WO"])
    res = run_bass_kernel_spmd(nc, in_maps, list(range(N_CORES)),
                               trace=trace)
    acc = np.zeros((T, D), dtype=np.float64)
    for c in range(N_CORES):
        acc += res.results[c]["out"].astype(np.float64)
    return acc.astype(np.float32)[None], res


def kernel(x, WQ, WK, WV, WO):
    out, _ = run_on_hw({"x": x, "WQ": WQ, "WK": WK, "WV": WV, "WO": WO})
    return out
